# revision 11
# baseline (speedup 1.0000x reference)
"""XNOR-Net++ style binary double-conv forward for Trainium2, 8 NeuronCores.

Computes, for x:[32,256,56,56] f32, w1,w2:[256,256,3,3] f32:
    alpha = (mean|w1| + mean|w2|)/2 over (I,kh,kw)            -> [O]
    out   = (conv(sign(x), sign(w1)) + conv(sign(x), sign(w2))) * alpha

Key algebraic fold: conv(s, sign(w1)) + conv(s, sign(w2)) ==
conv(s, sign(w1)+sign(w2)); the combined weight is in {-2,0,2} and the
activations in {-1,0,1}, both exact in fp8e4, so the conv's integer
part is reproduced exactly and only the final alpha multiply rounds.

Implementation: 9 shifted-tap matmuls accumulate each output tile in
PSUM. fp8 + DoubleRow folds the K=256 contraction (2x128 C_in chunks)
into single matmuls. The padded sign image is stored flat ([58*58] per
C_in chunk, 58-wide rows, one guard byte on each end) so every tap
window is a contiguous 464-element slice; the wrap-around columns land
in the two padding columns of each 8-row output block and are dropped
by the PSUM->SBUF scale-copy.

Scheduling: x loads are software-prefetched two images ahead on the
sync/scalar HWDGE queues, weight loads go first, output DMAs alternate
sync/gpsimd so input prefetches never queue behind them. Padding
borders are zeroed with three small strided memsets per C_in plane
instead of a full-tile memset.

Sharding: data-parallel over batch, 4 images per core, weights
replicated; outputs concatenated on host.
"""

import numpy as np

P = 128
H = W = 56
WP = 58  # padded row width
PLANE = WP * WP  # 3364 flat padded plane
PLANE_STRIDE = 3376  # 16-aligned (DoubleRow AP step%16==0), >= 1+PLANE+1
NIMG = 4  # images per core
NCORES = 8
RB = 8  # output rows per matmul block
NBLK = H // RB  # 7
N_FREE = RB * WP  # 464 <= 512 (one PSUM bank)

_CACHE = {}


def _build_program():
    from contextlib import ExitStack

    import concourse.bacc as bacc
    import concourse.mybir as mybir
    import concourse.tile as tile
    from concourse.masks import make_identity

    dt = mybir.dt
    AF = mybir.ActivationFunctionType

    nc = bacc.Bacc(
        "TRN2",
        target_bir_lowering=False,
        debug=False,
        num_devices=NCORES,
    )
    x = nc.dram_tensor("x", [NIMG, 256, H, W], dt.float32, kind="ExternalInput").ap()
    w1 = nc.dram_tensor("w1", [256, 256, 3, 3], dt.float32, kind="ExternalInput").ap()
    w2 = nc.dram_tensor("w2", [256, 256, 3, 3], dt.float32, kind="ExternalInput").ap()
    out = nc.dram_tensor(
        "out", [NIMG, 256, H, W], dt.float32, kind="ExternalOutput"
    ).ap()

    with tile.TileContext(nc) as tc, ExitStack() as ctx:
        consts = ctx.enter_context(tc.tile_pool(name="consts", bufs=1))
        wprep = ctx.enter_context(tc.tile_pool(name="wprep", bufs=2))
        xraw_pool = ctx.enter_context(tc.tile_pool(name="xraw", bufs=4))
        xpad_pool = ctx.enter_context(tc.tile_pool(name="xpad", bufs=3))
        psum_pool = ctx.enter_context(tc.tile_pool(name="psum", bufs=7, space="PSUM"))
        outp = ctx.enter_context(tc.tile_pool(name="outp", bufs=4))

        ident = consts.tile([P, P], dt.bfloat16)
        make_identity(nc, ident)

        # alpha per output channel, one column per oc chunk
        alpha_sb = consts.tile([P, 2], dt.float32)
        # per-(oc,tap) stationary weight tiles [i_local, ic, o_local], fp8
        lhsT_t = [
            [
                consts.tile([P, 2, P], dt.float8e4, name=f"lhsT_{oc}_{tap}")
                for tap in range(9)
            ]
            for oc in range(2)
        ]

        # ---- weight DMAs first: small, on the critical path ----
        wr = [[None, None], [None, None]]
        for oc in range(2):
            wr1 = wprep.tile([P, 256, 3, 3], dt.float32, tag="wraw", name=f"wr1_{oc}")
            wr2 = wprep.tile([P, 256, 3, 3], dt.float32, tag="wraw2", name=f"wr2_{oc}")
            nc.sync.dma_start(out=wr1, in_=w1[oc * P : (oc + 1) * P])
            nc.scalar.dma_start(out=wr2, in_=w2[oc * P : (oc + 1) * P])
            wr[oc] = [wr1, wr2]

        # ---- x prefetch machinery ----
        xps = [None] * NIMG

        xrs = [[None, None] for _ in range(NIMG)]

        def load_dma(img, engines):
            xp = xpad_pool.tile([P, 2, PLANE_STRIDE], dt.float8e4, tag="xp", name=f"xp_{img}")
            xps[img] = xp
            for ic in range(2):
                # zero only the padding borders: [guard+top row], [bottom row
                # +tail guard], and the adjacent (right,left) pad pairs
                # between consecutive interior rows.
                nc.gpsimd.memset(xp[:, ic, 0:59], 0.0)
                nc.gpsimd.memset(xp[:, ic, 3306:PLANE_STRIDE], 0.0)
                pairs = xp[:, ic, 58 : 58 + 56 * WP].rearrange(
                    "p (r w) -> p r w", w=WP
                )[:, :, 0:2]
                nc.gpsimd.memset(pairs, 0.0)
                xr = xraw_pool.tile([P, H, W], dt.float32, tag="xr", name=f"xr_{img}_{ic}")
                engines[ic].dma_start(out=xr, in_=x[img, ic * P : (ic + 1) * P])
                xrs[img][ic] = xr

        def sign_img(img):
            xp = xps[img]
            for ic in range(2):
                interior = xp[:, ic, 1 : 1 + PLANE].rearrange(
                    "p (h w) -> p h w", w=WP
                )[:, 1 : 1 + H, 1 : 1 + W]
                nc.scalar.activation(out=interior, in_=xrs[img][ic], func=AF.Sign)

        def sign_weights(oc):
            wr1, wr2 = wr[oc]
            ws1 = wprep.tile(
                [P, 256, 3, 3], dt.bfloat16, tag="wsign", name=f"ws1_{oc}"
            )
            ws2 = wprep.tile(
                [P, 256, 3, 3], dt.bfloat16, tag="wsign2", name=f"ws2_{oc}"
            )
            nc.scalar.activation(out=ws1, in_=wr1, func=AF.Sign)
            nc.scalar.activation(out=ws2, in_=wr2, func=AF.Sign)
            wsum = wprep.tile([P, 256, 3, 3], dt.bfloat16, tag="wsum", name=f"wsum_{oc}")
            nc.vector.tensor_add(out=wsum, in0=ws1, in1=ws2)
            return wsum

        def transpose_weights(oc, wsum):
            for tap in range(9):
                ky, kx = tap // 3, tap % 3
                for ic in range(2):
                    pt = psum_pool.tile(
                        [P, P], dt.bfloat16, tag="acc", name=f"wt_{oc}_{tap}_{ic}"
                    )
                    nc.tensor.transpose(
                        pt, wsum[:, ic * P : (ic + 1) * P, ky, kx], ident
                    )
                    nc.vector.tensor_copy(out=lhsT_t[oc][tap][:, ic, :], in_=pt)

        def alpha_reduce(oc):
            wr1, wr2 = wr[oc]
            asum1 = wprep.tile([P, 1], dt.float32, tag="asum", name=f"as1_{oc}")
            asum2 = wprep.tile([P, 1], dt.float32, tag="asum2", name=f"as2_{oc}")
            for asum, w_ in ((asum1, wr1), (asum2, wr2)):
                nc.vector.tensor_reduce(
                    out=asum,
                    in_=w_[:].rearrange("p a b c -> p (a b c)"),
                    axis=mybir.AxisListType.X,
                    op=mybir.AluOpType.add,
                    apply_absolute_value=True,
                )
            nc.vector.tensor_add(out=alpha_sb[:, oc : oc + 1], in0=asum1, in1=asum2)
            nc.vector.tensor_scalar_mul(
                alpha_sb[:, oc : oc + 1], alpha_sb[:, oc : oc + 1], 1.0 / (2 * 2304)
            )

        def conv_oc(img, oc):
            xp = xps[img]
            psums = []
            for blk in range(NBLK):
                psums.append(
                    psum_pool.tile(
                        [P, N_FREE], dt.float32, tag="acc", name=f"acc_{img}_{oc}_{blk}"
                    )
                )
            for tap in range(9):
                ky, kx = tap // 3, tap % 3
                lhsT = lhsT_t[oc][tap]
                for blk in range(NBLK):
                    win = (blk * RB + ky) * WP + kx
                    nc.tensor.matmul(
                        out=psums[blk],
                        lhsT=lhsT,
                        rhs=xp[:, :, win : win + N_FREE],
                        start=(tap == 0),
                        stop=(tap == 8),
                        perf_mode=mybir.MatmulPerfMode.DoubleRow,
                    )
            for blk in range(NBLK):
                rs = blk * RB
                ot = outp.tile([P, RB, W], dt.float32, tag="ot", name=f"ot_{img}_{oc}_{blk}")
                psv = psums[blk][:].rearrange("p (h w) -> p h w", w=WP)[:, :, 1 : 1 + W]
                if blk % 2 == 0:
                    nc.vector.tensor_scalar_mul(ot, psv, alpha_sb[:, oc : oc + 1])
                else:
                    nc.scalar.activation(
                        out=ot, in_=psv, func=AF.Copy, scale=alpha_sb[:, oc : oc + 1]
                    )
                dma_eng = nc.sync if blk % 2 == 0 else nc.gpsimd
                dma_eng.dma_start(
                    out=out[img, oc * P : (oc + 1) * P, rs : rs + RB, :], in_=ot
                )

        # ---- schedule ----
        # startup: weight DMAs already queued first on sync/scalar;
        # img0 x rides the gpsimd SWDGE queue, img1 behind the weights.
        load_dma(0, (nc.gpsimd, nc.gpsimd))
        load_dma(1, (nc.gpsimd, nc.gpsimd))
        wsum0 = sign_weights(0)
        wsum1 = sign_weights(1)
        transpose_weights(0, wsum0)
        transpose_weights(1, wsum1)
        sign_img(0)
        sign_img(1)
        alpha_reduce(0)
        alpha_reduce(1)
        conv_oc(0, 0)
        load_dma(2, (nc.sync, nc.scalar))
        sign_img(2)
        conv_oc(0, 1)
        conv_oc(1, 0)
        load_dma(3, (nc.sync, nc.scalar))
        sign_img(3)
        conv_oc(1, 1)
        conv_oc(2, 0)
        conv_oc(2, 1)
        conv_oc(3, 0)
        conv_oc(3, 1)

    nc.compile()
    return nc


def _get_program():
    if "nc" not in _CACHE:
        _CACHE["nc"] = _build_program()
    return _CACHE["nc"]


def _run(x, weight1, weight2, **spmd_kwargs):
    from concourse.bass_utils import run_bass_kernel_spmd

    nc = _get_program()
    x = np.ascontiguousarray(x, dtype=np.float32)
    w1 = np.ascontiguousarray(weight1, dtype=np.float32)
    w2 = np.ascontiguousarray(weight2, dtype=np.float32)
    in_maps = [
        {"x": x[i * NIMG : (i + 1) * NIMG], "w1": w1, "w2": w2} for i in range(NCORES)
    ]
    res = run_bass_kernel_spmd(nc, in_maps, list(range(NCORES)), **spmd_kwargs)
    out = np.concatenate([res.results[i]["out"] for i in range(NCORES)], axis=0)
    return out, res


def kernel(x, weight1, weight2):
    out, _ = _run(x, weight1, weight2)
    return out
